# revision 8
# baseline (speedup 1.0000x reference)
"""Multi-head attention (B=4,N=2048,C=768,H=12) on 8 trn2 NeuronCores.

Sharding: data-parallel over B (4 batches x 2 cores each), tensor-parallel
over heads (6 heads per core). Each core:
  - QKV projection for its 6 heads (bf16 matmuls with FWL, fp32 accumulate)
  - transposed scores st[kv, q]: two heads row-packed into PE partitions
    0-63 / 64-127 -> the two matmuls run concurrently as PE row-tiles
  - exp on ScalarE (scale fused), bf16 output
  - attn@V with a ones-column appended to V (M=65) so the softmax
    denominator falls out of the same matmul
  - normalize: reciprocal_approx_fast on the denominator row, gpsimd
    partition_broadcast, DVE multiply
  - output projection per query-strip (bf16, contraction 128 per
    head-pair) -> per-pair partial y (bf16) to DRAM
Host sums the six partials per batch (3 pairs x 2 cores) and adds the bias.
"""

import sys

import numpy as np
import ml_dtypes

_REPO = "/opt/trn_rl_repo"
if _REPO not in sys.path:
    sys.path.insert(0, _REPO)

import concourse.bacc as bacc
import concourse.mybir as mybir
import concourse.tile as tile
from concourse.bass_utils import run_bass_kernel_spmd

B, N, C, H, D = 4, 2048, 768, 12, 64
HL = H // 2          # heads per core
SCALE = D ** -0.5
NCORES = 8
KT_C = C // 128      # 6 contraction tiles over C
MT_QK = 2 * HL // 2  # 6 output tiles for q+k (3 Q pairs, 3 K pairs)
QS = N // 512        # 4 query strips
KVT = N // 128       # 16 kv tiles

F32 = mybir.dt.float32
BF16 = mybir.dt.bfloat16
EXP = mybir.ActivationFunctionType.Exp

_CACHE = {}


def _build():
    nc = bacc.Bacc("TRN2", target_bir_lowering=False, debug=False,
                   num_devices=NCORES)
    xT = nc.dram_tensor("xT", [C, N], BF16, kind="ExternalInput").ap()
    wqkT = nc.dram_tensor("wqkT", [C, 2 * HL * D], BF16, kind="ExternalInput").ap()
    wvT = nc.dram_tensor("wvT", [C, HL * D], BF16, kind="ExternalInput").ap()
    wpT = nc.dram_tensor("wpT", [HL * D, C], BF16, kind="ExternalInput").ap()
    y = nc.dram_tensor("y", [HL // 2, N, C], BF16, kind="ExternalOutput").ap()

    with tile.TileContext(nc) as tc:
        with (
            tc.tile_pool(name="singles", bufs=1) as singles,
            tc.tile_pool(name="ps_a", bufs=2, space="PSUM") as ps_a,
            tc.tile_pool(name="ps_st", bufs=2, space="PSUM") as ps_st,
            tc.tile_pool(name="ps_out", bufs=2, space="PSUM") as ps_out,
            tc.tile_pool(name="est", bufs=6) as est_p,
            tc.tile_pool(name="rec", bufs=4) as rec_p,
            tc.tile_pool(name="rb", bufs=4) as rb_p,
            tc.tile_pool(name="ysb", bufs=5) as ysb_p,
        ):
            xT_sb = singles.tile([128, KT_C, N], BF16)
            wqk_sb = singles.tile([128, KT_C, 2 * HL * D], BF16)
            wv_sb = singles.tile([128, KT_C, HL * D], BF16)
            wp_sb = singles.tile([128, HL // 2, C], BF16)
            qk_sb = singles.tile([128, MT_QK, N], BF16)
            v_sb = singles.tile([128, KVT, HL, D + 1], BF16)
            # attention output in proj-ready pair layout: [128, pair, N]
            # (even head -> partitions 0-63 via DVE, odd head staged at 0-63
            # then DMA-shifted to partitions 64-127)
            attn_sb = singles.tile([128, HL // 2, N], BF16)

            # x strips qs-major so the first qk accumulation chain's inputs
            # land earliest; weight blocks interleaved
            for qs in range(QS):
                for kt in range(KT_C):
                    nc.sync.dma_start(
                        xT_sb[:, kt, qs * 512:(qs + 1) * 512],
                        xT[kt * 128:(kt + 1) * 128, qs * 512:(qs + 1) * 512])
                if qs == 0:
                    for kt in range(KT_C):
                        nc.sync.dma_start(wqk_sb[:, kt, :],
                                          wqkT[kt * 128:(kt + 1) * 128, :])
            for kt in range(KT_C):
                nc.sync.dma_start(wv_sb[:, kt, :], wvT[kt * 128:(kt + 1) * 128, :])
            for p in range(HL // 2):
                nc.sync.dma_start(wp_sb[:, p, :], wpT[p * 128:(p + 1) * 128, :])
            nc.vector.memset(v_sb[:, :, :, D:D + 1], 1.0)

            # warm the ACT exp table set during the DMA fill so the ~2.7us
            # ACT_TABLE_LOAD is off the first real exp's critical path
            warm_in = rec_p.tile([1, 2], F32, tag="warm")
            warm_out = rec_p.tile([1, 2], BF16, tag="warmo")
            nc.vector.memset(warm_in, 0.0)
            nc.scalar.activation(warm_out, warm_in, EXP, scale=SCALE)

            # PE is otherwise idle until the first xT tiles land;
            # dependency-free dummy matmuls fill that window and keep the
            # HAM clock-gate warm so the first real matmuls run at 2.4GHz
            nc.vector.memset(attn_sb[:, 0, 0:640], 0.0)
            for _ in range(24):
                warm_ps = ps_out.tile([128, 512], F32, tag="out")
                nc.tensor.matmul(warm_ps, lhsT=attn_sb[:, 0, 0:128],
                                 rhs=attn_sb[:, 0, 128:640])

            def emit_qk_strip(t, qs, pools=None):
                pool, tag = (pools[qs % len(pools)] if pools
                             else (ps_a, "ps_a"))
                ps = pool.tile([128, 512], F32, tag=tag)
                for kt in range(KT_C):
                    nc.tensor.matmul(
                        ps,
                        lhsT=wqk_sb[:, kt, t * 128:(t + 1) * 128],
                        rhs=xT_sb[:, kt, qs * 512:(qs + 1) * 512],
                        start=(kt == 0), stop=(kt == KT_C - 1),
                    )
                nc.vector.tensor_copy(qk_sb[:, t, qs * 512:(qs + 1) * 512], ps)

            def emit_qk_tile(t, pools=None):
                for qs in range(QS):
                    emit_qk_strip(t, qs, pools)

            def emit_v(mts, pools=None):
                for mt in mts:
                    pool, tag = (pools[mt % len(pools)] if pools
                                 else (ps_a, "ps_a"))
                    ps = pool.tile([128, HL * D], F32, tag=tag)
                    for kt in range(KT_C):
                        nc.tensor.matmul(
                            ps,
                            lhsT=xT_sb[:, kt, mt * 128:(mt + 1) * 128],
                            rhs=wv_sb[:, kt, :],
                            start=(kt == 0), stop=(kt == KT_C - 1),
                        )
                    nc.vector.tensor_copy(
                        v_sb[:, mt, :, 0:D],
                        ps.rearrange("p (h d) -> p h d", h=HL),
                    )

            def emit_proj_block(pr, mt):
                ysb = ysb_p.tile([128, 2, 384], BF16, tag="ysb")
                for ns in range(2):
                    yp = ps_a.tile([128, 384], F32, tag="ps_a")
                    nc.tensor.matmul(
                        yp,
                        lhsT=attn_sb[:, pr, mt * 128:(mt + 1) * 128],
                        rhs=wp_sb[:, pr, ns * 384:(ns + 1) * 384],
                    )
                    nc.vector.tensor_copy(ysb[:, ns, :], yp)
                    # split the y write per ns half so the last block's
                    # DMA tail is short
                    nc.sync.dma_start(
                        y[pr, mt * 128:(mt + 1) * 128,
                          ns * 384:(ns + 1) * 384],
                        ysb[:, ns, :])

            def emit_attention_strip(pr, qs, proj_pending=None, qk_next=()):
                tq, tk = pr, HL // 2 + pr
                qsl = slice(qs * 512, (qs + 1) * 512)
                out_a = ps_out.tile([128, 512], F32, tag="out")
                out_b = ps_out.tile([128, 512], F32, tag="out")
                outs = [out_a, out_b]
                # proj of the PREVIOUS strip and qk chains of the NEXT pair
                # are interleaved into this strip's kt loop: their matmuls
                # fill the PE idle while ScalarE paces the exp chain, and
                # the proj PSUM->SBUF casts land on DVE before this strip's
                # normalize chain
                proj_mts = [] if proj_pending is None else [
                    (proj_pending[0], 4 * proj_pending[1] + i)
                    for i in range(4)]
                qk_chains = list(qk_next)
                st_tiles = {}

                def do_st(kt):
                    # both heads' scores into one 2-bank tile, one exp;
                    # the two matmuls run concurrently as PE row-tiles
                    st = ps_st.tile([128, 2, 512], F32, tag="st")
                    for half in range(2):
                        p0, p1 = half * 64, (half + 1) * 64
                        nc.tensor.matmul(
                            st[:, half, :],
                            lhsT=qk_sb[p0:p1, tk, kt * 128:(kt + 1) * 128],
                            rhs=qk_sb[p0:p1, tq, qsl],
                        )
                    st_tiles[kt] = st

                # software pipeline: st(kt+1) is emitted before av(kt) so
                # the in-order PE keeps ScalarE fed one tile ahead
                do_st(0)
                for kt in range(KVT):
                    est = est_p.tile([128, 2, 512], BF16, tag="est")
                    nc.scalar.activation(est, st_tiles.pop(kt), EXP, scale=SCALE)
                    if kt + 1 < KVT:
                        do_st(kt + 1)
                    for half in range(2):
                        h = 2 * pr + half
                        nc.tensor.matmul(
                            outs[half][0:D + 1, :],
                            lhsT=v_sb[:, kt, h, :],
                            rhs=est[:, half, :],
                            start=(kt == 0), stop=(kt == KVT - 1),
                        )
                    if proj_mts and kt in (2, 5, 8, 11):
                        emit_proj_block(*proj_mts.pop(0))
                    if qk_chains and kt in (12, 14):
                        emit_qk_strip(*qk_chains.pop(0))
                # half B first: its chain ends with an extra SBUF-SBUF DMA
                # shift, so starting it first shortens the proj dependency
                for half in (1, 0):
                    out_ps = outs[half]
                    # stage to SBUF right away so the PSUM bank frees
                    # before the normalize chain runs
                    ostg = rec_p.tile([65, 512], F32, tag="ostg")
                    nc.vector.tensor_copy(ostg, out_ps[0:D + 1, :])
                    # the custom-DVE fast reciprocal mis-reads inputs whose
                    # base partition differs from the output's, so DMA the
                    # denominator row down to partition 0 first
                    den = rec_p.tile([1, 512], F32, tag="den")
                    nc.sync.dma_start(den, ostg[D:D + 1, :])
                    rec = rec_p.tile([1, 512], F32, tag="rec")
                    nc.vector.reciprocal_approx_fast(rec, den)
                    rb = rb_p.tile([64, 512], F32, tag="rb")
                    nc.gpsimd.partition_broadcast(rb, rec)
                    if half == 0:
                        nc.vector.tensor_mul(
                            attn_sb[0:64, pr, qsl], ostg[0:D, :], rb)
                    else:
                        stg = rb_p.tile([64, 512], BF16, tag="astg")
                        nc.vector.tensor_mul(stg, ostg[0:D, :], rb)
                        # shift odd head into partitions 64-127
                        nc.sync.dma_start(attn_sb[64:128, pr, qsl], stg)
                assert not proj_mts and not qk_chains

            def emit_proj_strip(pr, qs):
                for mt in range(4 * qs, 4 * qs + 4):
                    emit_proj_block(pr, mt)

            # Emission order = scheduler priority. Attention (ACT-bound)
            # leads; qk strips for the NEXT pair and proj for the current
            # strip are emitted after it so the in-order PE slots them into
            # the gaps where it waits on ScalarE's exp.
            # pre-attention phase may borrow the (idle) attention PSUM
            # slots so more accumulation groups overlap the xT DMA fill
            fill_pools = [(ps_a, "ps_a"), (ps_st, "st"), (ps_out, "out")]
            emit_qk_tile(0, fill_pools)
            emit_qk_tile(HL // 2, fill_pools)
            emit_v(range(KVT), fill_pools)
            # proj is delayed by one strip: the normalize chain of strip i
            # finishes under strip i+1's attention matmuls, so the in-order
            # PE never stalls on it (except the final tail strip)
            pending = None
            for pr in range(HL // 2):
                for qs in range(QS):
                    qk_next = ((pr + 1, qs), (HL // 2 + pr + 1, qs)) \
                        if pr + 1 < HL // 2 else ()
                    emit_attention_strip(pr, qs, proj_pending=pending,
                                         qk_next=qk_next)
                    pending = (pr, qs)
            emit_proj_strip(*pending)

    nc.compile()
    return nc


def _get_nc():
    if "nc" not in _CACHE:
        _CACHE["nc"] = _build()
    return _CACHE["nc"]


def _prep_inputs(x, w_qkv, w_proj):
    """Per-core input dicts. Core c: batch c//2, head-half c%2."""
    wq, wk, wv = w_qkv[0:C], w_qkv[C:2 * C], w_qkv[2 * C:3 * C]
    in_maps = []
    for core in range(NCORES):
        b, p = divmod(core, 2)
        heads = [p * HL + j for j in range(HL)]
        qk_rows = np.concatenate(
            [wq[h * D:(h + 1) * D] for h in heads]
            + [wk[h * D:(h + 1) * D] for h in heads], axis=0)   # [768, C]
        v_rows = np.concatenate(
            [wv[h * D:(h + 1) * D] for h in heads], axis=0)     # [384, C]
        p_cols = np.concatenate(
            [w_proj[:, h * D:(h + 1) * D] for h in heads], axis=1)  # [C, 384]
        in_maps.append({
            "xT": np.ascontiguousarray(x[b].T).astype(ml_dtypes.bfloat16),
            "wqkT": np.ascontiguousarray(qk_rows.T).astype(ml_dtypes.bfloat16),
            "wvT": np.ascontiguousarray(v_rows.T).astype(ml_dtypes.bfloat16),
            "wpT": np.ascontiguousarray(p_cols.T).astype(ml_dtypes.bfloat16),
        })
    return in_maps


def kernel(x, w_qkv, w_proj, b_proj, _trace=False):
    x = np.asarray(x, dtype=np.float32)
    w_qkv = np.asarray(w_qkv, dtype=np.float32)
    w_proj = np.asarray(w_proj, dtype=np.float32)
    b_proj = np.asarray(b_proj, dtype=np.float32)

    nc = _get_nc()
    in_maps = _prep_inputs(x, w_qkv, w_proj)
    # retry: transient NRT_EXEC_UNIT_UNRECOVERABLE has been observed once
    # on a first attempt and succeeded immediately on retry
    last_exc = None
    for _attempt in range(3):
        try:
            res = run_bass_kernel_spmd(nc, in_maps,
                                       core_ids=list(range(NCORES)),
                                       trace=_trace)
            break
        except Exception as e:
            last_exc = e
    else:
        raise last_exc
    _CACHE["last_results"] = res

    out = np.empty((B, N, C), dtype=np.float32)
    for b in range(B):
        out[b] = (res.results[2 * b]["y"].astype(np.float32).sum(0)
                  + res.results[2 * b + 1]["y"].astype(np.float32).sum(0)
                  + b_proj)
    return out
